# revision 1
# baseline (speedup 1.0000x reference)
"""Local causal (sliding-window) attention on 8 Trainium2 NeuronCores.

Strategy: sequence-parallel. Each core owns 512 consecutive query tokens of
one batch element (cores 0-3 -> batch 0, cores 4-7 -> batch 1) plus a
128-token halo of preceding tokens whose k/v are recomputed locally, so no
inter-core communication is needed. The dataflow is fully transposed
(features on partitions, tokens on the free dim) so no on-device transposes
are required: the host pre-transposes each core's x-shard and transposes the
per-core output back while gathering. All matmuls run in float32r (fp22) at
full rate.

Attention per (supertile st of 256 queries, head h): the 256-query window
spans 3 key blocks r0/r1/r2 of 128 tokens. Masks are DVE-copied into PSUM
first and the QK matmuls accumulate onto them (start=False); the fully
masked quadrants of r0/r2 are never computed (half-width matmuls). exp runs
on ScalarE into SBUF p-tiles; AV accumulates v^T p with an extra ones
column in v producing the softmax denominators, which take a DMA round trip
through a 16-partition tile for one batched reciprocal, then a GpSimd
partition-broadcast feeds the normalizing multiply.
"""

import sys

sys.path.insert(0, "/opt/trn_rl_repo")
import numpy as np

B, S, D = 2, 2048, 1024
H, DH = 16, 64
WINDOW = 128
NCORES = 8
SLOC = 512            # queries per core
HALO = 128
TLOC = SLOC + HALO    # 640 local tokens (halo + queries)
NST = 2               # query supertiles of 256 per core
CPB = NCORES // B     # cores per batch element

_cached = {}


def _build():
    import concourse.bacc as bacc
    import concourse.mybir as mybir
    import concourse.tile as tile

    f32 = mybir.dt.float32
    f32r = mybir.dt.float32r
    bf16 = mybir.dt.bfloat16
    AF = mybir.ActivationFunctionType

    nc = bacc.Bacc(None)
    xT_d = nc.declare_dram_parameter("xT", [D, TLOC], f32, isOutput=False)
    wqkv_d = nc.declare_dram_parameter("w_qkv", [D, 3 * D], f32, isOutput=False)
    wout_d = nc.declare_dram_parameter("w_out", [D, D], f32, isOutput=False)
    mask_d = nc.declare_dram_parameter("mask", [NST, 128, 512], f32, isOutput=False)
    eye_d = nc.declare_dram_parameter("eye", [128, 128], f32, isOutput=False)
    ones_d = nc.declare_dram_parameter("ones", [128, H], f32, isOutput=False)
    out_d = nc.declare_dram_parameter("outT", [D, SLOC], f32, isOutput=True)

    with tile.TileContext(nc) as tc:
        with (
            tc.tile_pool(name="sb", bufs=1) as sb,
            tc.tile_pool(name="qkps", bufs=1, space="PSUM") as qkps,
            tc.tile_pool(name="scps", bufs=1, space="PSUM") as scps,
            tc.tile_pool(name="aops", bufs=1, space="PSUM") as aops,
        ):
            # ---- persistent SBUF tiles; DMA order: xt+ones, wq stream, masks, wout
            xt = [sb.tile([128, TLOC], f32r, tag=f"xt{k}", name=f"xt{k}") for k in range(8)]
            for k in range(8):
                nc.sync.dma_start(out=xt[k][:], in_=xT_d[k * 128:(k + 1) * 128, :].bitcast(f32r))
            ones_sb = sb.tile([128, H], f32r, tag="ones", name="ones_sb")
            nc.sync.dma_start(out=ones_sb[:], in_=ones_d[:].bitcast(f32r))

            qT = [sb.tile([128, SLOC], f32r, tag=f"qT{i}", name=f"qT{i}") for i in range(8)]
            kT = [sb.tile([128, TLOC], f32r, tag=f"kT{i}", name=f"kT{i}") for i in range(8)]
            vt = [sb.tile([128, 65 * H], f32r, tag=f"v{t}", name=f"v{t}") for t in range(5)]
            for t in range(5):
                v_ones = vt[t].rearrange("p (h c) -> p h c", c=65)[:, :, 64]
                nc.vector.tensor_copy(v_ones, ones_sb[:])
            att = [sb.tile([128, SLOC], f32r, tag=f"at{t}", name=f"at{t}") for t in range(8)]

            def wq_dma(cb):
                tiles = []
                for k in range(8):
                    wqk = sb.tile([128, 512], f32r, tag="wq", bufs=16, name=f"wq{cb}_{k}")
                    nc.sync.dma_start(
                        out=wqk[:],
                        in_=wqkv_d[k * 128:(k + 1) * 128, cb * 512:(cb + 1) * 512].bitcast(f32r),
                    )
                    tiles.append(wqk)
                return tiles

            # ---- phase 1: qkv projection ----
            for cb in range(2):            # q columns; queries only
                wq = wq_dma(cb)
                for m in range(4):
                    ps = qkps.tile([128, 512], f32, tag="qk", bufs=2, name=f"psq{cb}_{m}")
                    for k in range(8):
                        nc.tensor.matmul(
                            ps[:], wq[k][:, m * 128:(m + 1) * 128], xt[k][:, HALO:TLOC],
                            start=(k == 0), stop=(k == 7),
                        )
                    nc.scalar.copy(qT[cb * 4 + m][:], ps[:])
            for cb in range(2, 4):         # k columns; all 640 tokens
                wq = wq_dma(cb)
                for m in range(4):
                    for n in range(2):
                        ps = qkps.tile([128, 320], f32, tag="qk", bufs=2, name=f"psk{cb}_{m}_{n}")
                        for k in range(8):
                            nc.tensor.matmul(
                                ps[:], wq[k][:, m * 128:(m + 1) * 128],
                                xt[k][:, n * 320:(n + 1) * 320],
                                start=(k == 0), stop=(k == 7),
                            )
                        nc.scalar.copy(kT[(cb - 2) * 4 + m][:, n * 320:(n + 1) * 320], ps[:])
            # v columns: token-tile-major across both column halves so vt[t]
            # completes in jb order for the attention pipeline
            msk = [sb.tile([128, 512], f32r, tag=f"mk{i}", name=f"mk{i}") for i in range(NST)]
            eye_sb = sb.tile([128, 128], f32r, tag="eye", name="eye_sb")
            nc.sync.dma_start(out=eye_sb[:], in_=eye_d[:].bitcast(f32r))
            for st in range(NST):
                nc.sync.dma_start(out=msk[st][:], in_=mask_d[st].bitcast(f32r))
            wq4 = wq_dma(4)
            wq5 = wq_dma(5)
            for t in range(5):
                for half, wq in ((0, wq4), (1, wq5)):
                    ps = qkps.tile([128, 512], f32, tag="qk", bufs=2, name=f"psv{t}_{half}")
                    for k in range(8):
                        nc.tensor.matmul(
                            ps[:], xt[k][:, t * 128:(t + 1) * 128], wq[k][:, :],
                            start=(k == 0), stop=(k == 7),
                        )
                    h0 = half * 8
                    dst = vt[t].rearrange("p (h c) -> p h c", c=65)[:, h0:h0 + 8, 0:64]
                    src = ps[:].rearrange("p (h c) -> p h c", c=64)
                    nc.scalar.copy(dst, src)
            # w_out reuses the streamed-weight slots: half A = cols 0:512 of row
            # block k (proj m 0..3), half B = cols 512:1024 (m 4..7)
            woA = []
            woB = []
            for k in range(8):
                wa = sb.tile([128, 512], f32r, tag="wq", bufs=16, name=f"woA{k}")
                nc.sync.dma_start(out=wa[:], in_=wout_d[k * 128:(k + 1) * 128, 0:512].bitcast(f32r))
                woA.append(wa)
            for k in range(8):
                wb = sb.tile([128, 512], f32r, tag="wq", bufs=16, name=f"woB{k}")
                nc.sync.dma_start(out=wb[:], in_=wout_d[k * 128:(k + 1) * 128, 512:1024].bitcast(f32r))
                woB.append(wb)

            # ---- phase 2+3: attention and output projection ----
            DEPTH = 3
            scat = sb.tile([1, H * 256], f32, tag="scat", name="scat")
            rcat = sb.tile([1, H * 256], f32, tag="rcat", name="rcat")
            for st in range(NST):
                q0 = st * 256
                pend = {}

                def emit_qk(h, st=st, q0=q0, pend=pend):
                    t, poff = h // 2, (h % 2) * 64
                    jb = st * 2
                    sc = scps.tile([128, 512], f32, tag="sc", bufs=4, name=f"sc_{st}_{h}")
                    nc.tensor.matmul(
                        sc[:], eye_sb[:], msk[st][:],
                        start=True, stop=False, skip_group_check=True,
                    )
                    nc.tensor.matmul(
                        sc[:, 0:128],
                        kT[t][poff:poff + 64, jb * 128:(jb + 1) * 128],
                        qT[t][poff:poff + 64, q0:q0 + 128],
                        start=False, stop=False, skip_group_check=True,
                    )
                    nc.tensor.matmul(
                        sc[:, 128:256],
                        kT[t][poff:poff + 64, (jb + 2) * 128:(jb + 3) * 128],
                        qT[t][poff:poff + 64, q0 + 128:q0 + 256],
                        start=False, stop=False, skip_group_check=True,
                    )
                    nc.tensor.matmul(
                        sc[:, 256:512],
                        kT[t][poff:poff + 64, (jb + 1) * 128:(jb + 2) * 128],
                        qT[t][poff:poff + 64, q0:q0 + 256],
                        start=False, stop=True, skip_group_check=True,
                    )
                    p = sb.tile([128, 512], f32r, tag="pp", bufs=DEPTH + 3, name=f"p_{st}_{h}")
                    nc.scalar.activation(p[:], sc[:], AF.Exp, scale=0.125)
                    pend[h] = p

                def emit_av(h, st=st, q0=q0, pend=pend):
                    t, poff = h // 2, (h % 2) * 64
                    jb = st * 2
                    p = pend.pop(h)
                    av = aops.tile([65, 256], f32, tag="ao", bufs=2, name=f"av{st}_{h}")
                    nc.tensor.matmul(
                        av[:], vt[jb + 1][:, h * 65:h * 65 + 65], p[:, 256:512],
                        start=True, stop=False, skip_group_check=True,
                    )
                    nc.tensor.matmul(
                        av[:, 0:128], vt[jb][:, h * 65:h * 65 + 65], p[:, 0:128],
                        start=False, stop=False, skip_group_check=True,
                    )
                    nc.tensor.matmul(
                        av[:, 128:256], vt[jb + 2][:, h * 65:h * 65 + 65], p[:, 128:256],
                        start=False, stop=True, skip_group_check=True,
                    )
                    nc.scalar.copy(scat[0:1, h * 256:(h + 1) * 256], av[64:65, :])
                    nc.vector.tensor_copy(att[t][poff:poff + 64, q0:q0 + 256], av[0:64, :])

                for step in range(H + DEPTH):
                    if step < H:
                        emit_qk(step)
                    if step >= DEPTH:
                        emit_av(step - DEPTH)

                # batched softmax denominators
                s16 = sb.tile([16, 256], f32, tag="s16", bufs=2, name=f"s16_{st}")
                for h in range(H):
                    nc.sync.dma_start(out=s16[h:h + 1, :], in_=scat[0:1, h * 256:(h + 1) * 256])
                r16 = sb.tile([16, 256], f32, tag="r16", bufs=2, name=f"r16_{st}")
                nc.vector.reciprocal(r16[:], s16[:])
                for h in range(H):
                    nc.sync.dma_start(out=rcat[0:1, h * 256:(h + 1) * 256], in_=r16[h:h + 1, :])
                for h in range(H):
                    t, poff = h // 2, (h % 2) * 64
                    rb = sb.tile([128, 256], f32, tag="rb", bufs=4, name=f"rb{st}_{h}")
                    nc.gpsimd.partition_broadcast(rb[:], rcat[0:1, h * 256:(h + 1) * 256])
                    asl = att[t][poff:poff + 64, q0:q0 + 256]
                    nc.vector.tensor_mul(asl, asl, rb[poff:poff + 64, :])
                # output projection for this supertile
                for m in range(8):
                    wo = woA if m < 4 else woB
                    mc = (m % 4) * 128
                    po = aops.tile([128, 256], f32, tag="ao", bufs=2, name=f"po{st}_{m}")
                    for k in range(8):
                        nc.tensor.matmul(
                            po[:], wo[k][:, mc:mc + 128], att[k][:, q0:q0 + 256],
                            start=(k == 0), stop=(k == 7),
                        )
                    ot = sb.tile([128, 256], f32, tag="ot", bufs=4, name=f"ot{st}_{m}")
                    nc.scalar.copy(ot[:], po[:])
                    nc.sync.dma_start(
                        out=out_d[m * 128:(m + 1) * 128, q0:q0 + 256], in_=ot[:],
                    )

    nc.finalize()
    return nc


def _get_nc():
    if "nc" not in _cached:
        _cached["nc"] = _build()
    return _cached["nc"]


def _core_inputs(x, w_qkv, w_out):
    in_maps = []
    for c in range(NCORES):
        b, qs = c // CPB, (c % CPB) * SLOC
        xs = np.zeros((TLOC, D), dtype=np.float32)
        lo = max(0, qs - HALO)
        xs[HALO - (qs - lo):] = x[b, lo:qs + SLOC]
        # masks: additive bias on raw scores (exp applies the 0.125 scale).
        # mask[st][0] packs [r0 x queries 0:128 | r2 x queries 128:256];
        # mask[st][1] is r1 (middle key block) for all 256 queries.
        i = np.arange(256)[None, None, None, :]
        j = np.arange(128)[None, None, :, None]
        st = np.arange(NST)[:, None, None, None]
        r = np.arange(3)[None, :, None, None]
        qg = qs + st * 256 + i
        kg = qs + st * 256 - HALO + r * 128 + j
        allowed = (kg <= qg) & (kg > qg - WINDOW) & (kg >= 0)
        m3 = np.where(allowed, 0.0, -8e30).astype(np.float32)
        mask = np.empty((NST, 128, 512), dtype=np.float32)
        mask[:, :, 0:128] = m3[:, 0, :, 0:128]
        mask[:, :, 128:256] = m3[:, 2, :, 128:256]
        mask[:, :, 256:512] = m3[:, 1]
        in_maps.append(
            {
                "xT": np.ascontiguousarray(xs.T),
                "w_qkv": np.ascontiguousarray(w_qkv, dtype=np.float32),
                "w_out": np.ascontiguousarray(w_out, dtype=np.float32),
                "mask": mask,
                "ones": np.ones((128, H), dtype=np.float32),
                "eye": np.eye(128, dtype=np.float32),
            }
        )
    return in_maps


def kernel(x, w_qkv, w_out, _trace=False, _trace_kwargs=None):
    from concourse.bass_utils import run_bass_kernel_spmd

    x = np.asarray(x, dtype=np.float32)
    w_qkv = np.asarray(w_qkv, dtype=np.float32)
    w_out = np.asarray(w_out, dtype=np.float32)
    nc = _get_nc()
    in_maps = _core_inputs(x, w_qkv, w_out)
    res = run_bass_kernel_spmd(
        nc, in_maps, list(range(NCORES)), trace=_trace, **(_trace_kwargs or {})
    )
    out = np.concatenate(
        [res.results[c]["outT"].T for c in range(NCORES)], axis=0
    ).reshape(B, S, D)
    if _trace:
        return out, res
    return out



# revision 8
# speedup vs baseline: 1.4782x; 1.4782x over previous
"""Local causal (sliding-window) attention on 8 Trainium2 NeuronCores.

Strategy: sequence-parallel, fully transposed dataflow (features on
partitions, tokens on the free dim). Each core owns 512 consecutive query
tokens of one batch element (cores 0-3 -> batch 0, cores 4-7 -> batch 1)
plus a 128-token halo whose k/v are recomputed locally, so no inter-core
communication is needed.

All matmul inputs are bfloat16 (converted on host): halves HBM traffic vs
f32, runs the 128-wide attention matmuls at full PE rate (f32r drops to 1/4
rate below 256 moving columns), and enables fast weight load. PSUM
accumulation stays f32; softmax reciprocals are computed in f32.

Attention per (supertile st of 256 queries, head h): the 256-query window
spans 3 key blocks of 128 tokens; the fully-masked quadrants are never
computed (half-width matmuls). The sliding-window mask is applied
multiplicatively on the vector engine after the exp (instead of the
baseline's additive-bias matmul, which burned 512 PE cycles per head).
The AV matmul carries an extra ones column in v producing the softmax
denominators; per-head denominator rows are gathered by ScalarE copies
into a [16,256] tile, one batched DVE reciprocal per supertile, then a
one-hot-selector K=16 matmul broadcasts each head's reciprocal row across
partitions for the normalizing multiply (no DMA round trip, no gpsimd).
"""

import sys

sys.path.insert(0, "/opt/trn_rl_repo")
import numpy as np
import ml_dtypes

BF16 = ml_dtypes.bfloat16

B, S, D = 2, 2048, 1024
H, DH = 16, 64
WINDOW = 128
NCORES = 8
SLOC = 512            # queries per core
HALO = 128
TLOC = SLOC + HALO    # 640 local tokens (halo + queries)
NST = 2               # query supertiles of 256 per core
CPB = NCORES // B     # cores per batch element

_cached = {}


def _build():
    import concourse.bacc as bacc
    import concourse.mybir as mybir
    import concourse.tile as tile

    f32 = mybir.dt.float32
    bf16 = mybir.dt.bfloat16
    AF = mybir.ActivationFunctionType

    nc = bacc.Bacc(None)
    xT_d = nc.declare_dram_parameter("xT", [D, TLOC], bf16, isOutput=False)
    wqkv_d = nc.declare_dram_parameter("w_qkv", [D, 3 * D], bf16, isOutput=False)
    wout_d = nc.declare_dram_parameter("w_out", [D, D], bf16, isOutput=False)
    mask_d = nc.declare_dram_parameter("mask", [NST, 128, 512], bf16, isOutput=False)
    sel_d = nc.declare_dram_parameter("sel", [16, 16 * 128], bf16, isOutput=False)
    out_d = nc.declare_dram_parameter("outT", [D, SLOC], f32, isOutput=True)

    with tile.TileContext(nc) as tc:
        with (
            tc.tile_pool(name="sb", bufs=1) as sb,
            tc.tile_pool(name="qkps", bufs=1, space="PSUM") as qkps,
            tc.tile_pool(name="scps", bufs=1, space="PSUM") as scps,
            tc.tile_pool(name="aops", bufs=1, space="PSUM") as aops,
        ):
            # ---- resident SBUF tensors; DMA order = consumption order ----
            wqb = [sb.tile([128, 8 * 512], bf16, tag=f"wqb{cb}", name=f"wqb{cb}")
                   for cb in range(6)]
            xt = [sb.tile([128, TLOC], bf16, tag=f"xt{k}", name=f"xt{k}") for k in range(8)]

            def wq_dma(cb):
                nc.sync.dma_start(
                    out=wqb[cb].rearrange("p (k c) -> p k c", k=8),
                    in_=wqkv_d[:, cb * 512:(cb + 1) * 512].rearrange("(k p) c -> p k c", k=8),
                )

            wq_dma(0)
            for k in range(8):
                nc.sync.dma_start(out=xt[k][:], in_=xT_d[k * 128:(k + 1) * 128, :])
            for cb in range(1, 6):
                wq_dma(cb)
            msk = [sb.tile([128, 512], bf16, tag=f"mk{i}", name=f"mk{i}") for i in range(NST)]
            for st in range(NST):
                nc.sync.dma_start(out=msk[st][:], in_=mask_d[st])
            wo = sb.tile([128, 8 * 1024], bf16, tag="wo", name="wo")
            nc.sync.dma_start(
                out=wo.rearrange("p (k c) -> p k c", k=8),
                in_=wout_d.rearrange("(k p) c -> p k c", k=8),
            )

            qT = [sb.tile([128, SLOC], bf16, tag=f"qT{i}", name=f"qT{i}") for i in range(8)]
            kT = [sb.tile([128, TLOC], bf16, tag=f"kT{i}", name=f"kT{i}") for i in range(8)]
            vt = [sb.tile([128, 65 * H], bf16, tag=f"v{t}", name=f"v{t}") for t in range(5)]
            att = [sb.tile([128, SLOC], bf16, tag=f"at{t}", name=f"at{t}") for t in range(8)]
            for t in range(5):
                nc.vector.memset(vt[t].rearrange("p (h c) -> p h c", c=65)[:, :, 64], 1.0)
            # one-hot selector: sel[k, h*128+j] = (k == h); broadcasts row h of
            # the reciprocal tile across 128 partitions via a K=16 matmul
            sel = sb.tile([16, 16 * 128], bf16, tag="sel", name="sel")
            nc.sync.dma_start(out=sel[:], in_=sel_d[:])
            scat = [sb.tile([1, H * 256], f32, tag=f"scat{st}", name=f"scat{st}") for st in range(NST)]
            s16 = [sb.tile([16, 256], f32, tag=f"s16_{st}", name=f"s16_{st}") for st in range(NST)]
            r16f = [sb.tile([16, 256], f32, tag=f"r16f_{st}", name=f"r16f_{st}") for st in range(NST)]
            r16b = [sb.tile([16, 256], bf16, tag=f"r16b_{st}", name=f"r16b_{st}") for st in range(NST)]

            # ---- phase 1: qkv projection ----
            for cb in range(2):            # q columns; queries only
                for m in range(4):
                    ps = qkps.tile([128, 512], f32, tag="qk", bufs=2, name=f"psq{cb}_{m}")
                    for k in range(8):
                        nc.tensor.matmul(
                            ps[:], wqb[cb][:, k * 512 + m * 128:k * 512 + (m + 1) * 128],
                            xt[k][:, HALO:TLOC],
                            start=(k == 0), stop=(k == 7),
                        )
                    nc.scalar.copy(qT[cb * 4 + m][:], ps[:])
            for cb in range(2, 4):         # k columns; all 640 tokens
                for m in range(4):
                    for n in range(2):
                        ps = qkps.tile([128, 320], f32, tag="qk", bufs=2, name=f"psk{cb}_{m}_{n}")
                        for k in range(8):
                            nc.tensor.matmul(
                                ps[:], wqb[cb][:, k * 512 + m * 128:k * 512 + (m + 1) * 128],
                                xt[k][:, n * 320:(n + 1) * 320],
                                start=(k == 0), stop=(k == 7),
                            )
                        nc.scalar.copy(kT[(cb - 2) * 4 + m][:, n * 320:(n + 1) * 320], ps[:])
            # v: token-tile-major, x block stationary so tokens land on partitions
            for t in range(5):
                for half in range(2):
                    ps = qkps.tile([128, 512], f32, tag="qk", bufs=2, name=f"psv{t}_{half}")
                    for k in range(8):
                        nc.tensor.matmul(
                            ps[:], xt[k][:, t * 128:(t + 1) * 128],
                            wqb[4 + half][:, k * 512:(k + 1) * 512],
                            start=(k == 0), stop=(k == 7),
                        )
                    h0 = half * 8
                    dst = vt[t].rearrange("p (h c) -> p h c", c=65)[:, h0:h0 + 8, 0:64]
                    src = ps[:].rearrange("p (h c) -> p h c", c=64)
                    nc.scalar.copy(dst, src)

            # ---- phase 2+3: attention, then output projection ----
            DEPTH = 3
            pend = {}

            def emit_qk(st, h):
                t, poff = h // 2, (h % 2) * 64
                jb, q0 = st * 2, st * 256
                sc = scps.tile([128, 512], f32, tag="sc", bufs=2, name=f"sc_{st}_{h}")
                nc.tensor.matmul(
                    sc[:, 0:128],
                    kT[t][poff:poff + 64, jb * 128:(jb + 1) * 128],
                    qT[t][poff:poff + 64, q0:q0 + 128],
                    start=True, stop=True, skip_group_check=True,
                )
                nc.tensor.matmul(
                    sc[:, 128:256],
                    kT[t][poff:poff + 64, (jb + 2) * 128:(jb + 3) * 128],
                    qT[t][poff:poff + 64, q0 + 128:q0 + 256],
                    start=True, stop=True, skip_group_check=True,
                )
                nc.tensor.matmul(
                    sc[:, 256:512],
                    kT[t][poff:poff + 64, (jb + 1) * 128:(jb + 2) * 128],
                    qT[t][poff:poff + 64, q0:q0 + 256],
                    start=True, stop=True, skip_group_check=True,
                )
                p = sb.tile([128, 512], bf16, tag="pp", bufs=DEPTH + 3, name=f"p_{st}_{h}")
                nc.scalar.activation(p[:], sc[:], AF.Exp, scale=0.125)
                nc.vector.tensor_mul(p[:], p[:], msk[st][:])
                pend[(st, h)] = p

            def emit_av(st, h):
                t, poff = h // 2, (h % 2) * 64
                jb, q0 = st * 2, st * 256
                p = pend.pop((st, h))
                av = aops.tile([65, 256], f32, tag="ao", bufs=2, name=f"av{st}_{h}")
                nc.tensor.matmul(
                    av[:], vt[jb + 1][:, h * 65:h * 65 + 65], p[:, 256:512],
                    start=True, stop=False, skip_group_check=True,
                )
                nc.tensor.matmul(
                    av[:, 0:128], vt[jb][:, h * 65:h * 65 + 65], p[:, 0:128],
                    start=False, stop=False, skip_group_check=True,
                )
                nc.tensor.matmul(
                    av[:, 128:256], vt[jb + 2][:, h * 65:h * 65 + 65], p[:, 128:256],
                    start=False, stop=True, skip_group_check=True,
                )
                nc.scalar.copy(scat[st][0:1, h * 256:(h + 1) * 256], av[64:65, :])
                nc.sync.dma_start(
                    out=s16[st][h:h + 1, :], in_=scat[st][0:1, h * 256:(h + 1) * 256]
                )
                nc.vector.tensor_copy(att[t][poff:poff + 64, q0:q0 + 256], av[0:64, :])

            def emit_recip(st):
                nc.vector.reciprocal(r16f[st][:], s16[st][:])
                nc.scalar.copy(r16b[st][:], r16f[st][:])

            def emit_norm(st, h):
                t, poff = h // 2, (h % 2) * 64
                q0 = st * 256
                rb = aops.tile([128, 256], f32, tag="rb", bufs=2, name=f"rb{st}_{h}")
                nc.tensor.matmul(
                    rb[:], sel[:, h * 128:(h + 1) * 128], r16b[st][:],
                    start=True, stop=True, skip_group_check=True,
                )
                asl = att[t][poff:poff + 64, q0:q0 + 256]
                nc.vector.tensor_mul(asl, asl, rb[poff:poff + 64, :])

            def emit_outproj_m(st, m, ot_box):
                q0 = st * 256
                po = aops.tile([128, 256], f32, tag="ao", bufs=2, name=f"po{st}_{m}")
                for k in range(8):
                    nc.tensor.matmul(
                        po[:], wo[:, k * 1024 + m * 128:k * 1024 + (m + 1) * 128],
                        att[k][:, q0:q0 + 256],
                        start=(k == 0), stop=(k == 7),
                    )
                if m % 2 == 0:
                    ot_box[0] = sb.tile([128, 512], f32, tag="ot", bufs=3, name=f"ot{st}_{m}")
                    nc.scalar.copy(ot_box[0][:, 0:256], po[:])
                else:
                    ot = ot_box[0]
                    nc.scalar.copy(ot[:, 256:512], po[:])
                    nc.sync.dma_start(
                        out=out_d.rearrange("(m p) q -> p m q", m=8)[:, m - 1:m + 1, q0:q0 + 256],
                        in_=ot.rearrange("p (m q) -> p m q", m=2),
                    )

            # attention for both supertiles as one continuous 32-head pipeline;
            # st0's normalize is emitted inside st1's stream so the PE never
            # waits on the reciprocal chain
            for step in range(2 * H + DEPTH):
                if step < 2 * H:
                    emit_qk(step // H, step % H)
                if step >= DEPTH:
                    s = step - DEPTH
                    emit_av(s // H, s % H)
                    if s == H - 1:
                        emit_recip(0)
                ns = step - DEPTH - H - 2   # st0 normalizes, 2 heads per step
                if 0 <= ns < 8:
                    emit_norm(0, 2 * ns)
                    emit_norm(0, 2 * ns + 1)
            emit_recip(1)
            ot_box = [None]
            for m in range(8):             # st0 out-proj, st1 normalize interleaved
                emit_outproj_m(0, m, ot_box)
                emit_norm(1, 2 * m)
                emit_norm(1, 2 * m + 1)
            for m in range(8):
                emit_outproj_m(1, m, ot_box)

    nc.finalize()
    return nc


def _get_nc():
    if "nc" not in _cached:
        _cached["nc"] = _build()
    return _cached["nc"]


def _core_inputs(x, w_qkv, w_out):
    wq_b = np.ascontiguousarray(w_qkv.astype(BF16))
    wo_b = np.ascontiguousarray(w_out.astype(BF16))
    sel = np.zeros((16, 16 * 128), dtype=BF16)
    for h in range(H):
        sel[h, h * 128:(h + 1) * 128] = 1
    in_maps = []
    for c in range(NCORES):
        b, qs = c // CPB, (c % CPB) * SLOC
        xs = np.zeros((TLOC, D), dtype=np.float32)
        lo = max(0, qs - HALO)
        xs[HALO - (qs - lo):] = x[b, lo:qs + SLOC]
        # multiplicative 0/1 mask applied to exp(scores) on the DVE.
        # mask[st][:, 0:128] covers [r0 x queries 0:128], [:, 128:256] covers
        # [r2 x queries 128:256], [:, 256:512] is r1 for all 256 queries.
        i = np.arange(256)[None, None, None, :]
        j = np.arange(128)[None, None, :, None]
        st = np.arange(NST)[:, None, None, None]
        r = np.arange(3)[None, :, None, None]
        qg = qs + st * 256 + i
        kg = qs + st * 256 - HALO + r * 128 + j
        allowed = (kg <= qg) & (kg > qg - WINDOW) & (kg >= 0)
        m3 = allowed.astype(np.float32)
        mask = np.empty((NST, 128, 512), dtype=np.float32)
        mask[:, :, 0:128] = m3[:, 0, :, 0:128]
        mask[:, :, 128:256] = m3[:, 2, :, 128:256]
        mask[:, :, 256:512] = m3[:, 1]
        in_maps.append(
            {
                "xT": np.ascontiguousarray(xs.T.astype(BF16)),
                "w_qkv": wq_b,
                "w_out": wo_b,
                "mask": mask.astype(BF16),
                "sel": sel,
            }
        )
    return in_maps


def kernel(x, w_qkv, w_out, _trace=False, _trace_kwargs=None):
    from concourse.bass_utils import run_bass_kernel_spmd

    x = np.asarray(x, dtype=np.float32)
    w_qkv = np.asarray(w_qkv, dtype=np.float32)
    w_out = np.asarray(w_out, dtype=np.float32)
    nc = _get_nc()
    in_maps = _core_inputs(x, w_qkv, w_out)
    res = run_bass_kernel_spmd(
        nc, in_maps, list(range(NCORES)), trace=_trace, **(_trace_kwargs or {})
    )
    out = np.concatenate(
        [res.results[c]["outT"].T for c in range(NCORES)], axis=0
    ).reshape(B, S, D)
    if _trace:
        return out, res
    return out
